# revision 4
# baseline (speedup 1.0000x reference)
"""Causal self-attention (B=8, T=1024, C=1024, H=16) on 8 TRN2 NeuronCores.

Sharding: pure data-parallel over batch - core b computes batch element b
with fully replicated weights (B == n_cores, so no collectives needed).

v2 vs baseline:
  - x / Wqkv / Wproj are cast to bf16 on the HOST, so no on-device weight
    casts and half the weight DMA traffic.
  - x is transposed by the DMA xbar (dma_start_transpose) straight from
    DRAM into SBUF - no PE transposes, no PSUM evacuation copies.
  - QKV / V / proj matmul loops are ordered so consecutive matmuls share
    the stationary operand (halves LDWEIGHTS traffic).
  - Score matmuls for the head pair are emitted interleaved with explicit
    tile_position (0,0)/(64,0) so the PE can run both 64-contraction
    matmuls concurrently in different row groups.
  - Y evacuation split across ACT+DVE; V/proj-bias evacuations on Pool;
    softmax reciprocal chain batched per head with a DMA reblock; the
    last pair uses a DVE-recip + PE-broadcast fast path to cut the tail.
"""

import numpy as np
import ml_dtypes

import concourse.tile as tile
from concourse import bacc, mybir
from concourse.bass_utils import run_bass_kernel_spmd
from concourse.masks import make_identity

f32 = mybir.dt.float32
bf16 = mybir.dt.bfloat16
AF = mybir.ActivationFunctionType
ALU = mybir.AluOpType

B, T, C, H, HD = 8, 1024, 1024, 16, 64
P = 128
NT = T // P  # 8 token tiles
NS = C // P  # 8 contraction subtiles
W = 66  # per-head stride in V_sb: [64 vals][1 ones][1 pad]


def _build():
    nc = bacc.Bacc(trn_type="TRN2")
    x_d = nc.dram_tensor("x", (T, C), bf16, kind="ExternalInput")
    wqkv_d = nc.dram_tensor("wqkv", (C, 3 * C), bf16, kind="ExternalInput")
    bqkv_d = nc.dram_tensor("bqkv", (3 * C,), f32, kind="ExternalInput")
    wproj_d = nc.dram_tensor("wproj", (C, C), bf16, kind="ExternalInput")
    bproj_d = nc.dram_tensor("bproj", (C,), f32, kind="ExternalInput")
    out_d = nc.dram_tensor("out", (T, C), f32, kind="ExternalOutput")

    with tile.TileContext(nc) as tc:
        with (
            tc.tile_pool(name="big", bufs=1) as big,
            tc.tile_pool(name="ptp", bufs=4) as ptp,
            tc.tile_pool(name="small", bufs=3) as small,
            tc.tile_pool(name="small1", bufs=1) as small1,
            tc.tile_pool(name="outp", bufs=3) as outp,
            tc.tile_pool(name="dramp", bufs=6, space="DRAM") as dramp,
            tc.tile_pool(name="pmm", bufs=2, space="PSUM") as pmm,
        ):
            # ---------------- x transpose via DMA xbar ----------------
            # xt[p, s, t] = x[t, s*128+p]; single instruction, no PE work.
            xt_sb = big.tile([P, NS, T], bf16, tag="xt")
            nc.sync.dma_start_transpose(xt_sb, x_d[:, :])

            # ---------------- weights (bf16, no casts) ----------------
            wqkv_sb = big.tile([P, NS, 3 * C], bf16, tag="wqkv")
            wq_r = wqkv_d[:, :].rearrange("(s p) i -> p s i", p=P)
            # Q/K columns in m-pair slices so QK m=0 unblocks early
            for mp in range(NS):
                nc.sync.dma_start(
                    wqkv_sb[:, :, mp * 256 : (mp + 1) * 256],
                    wq_r[:, :, mp * 256 : (mp + 1) * 256],
                )
            # V columns per s-row (2KB descriptors)
            for s in range(NS):
                nc.sync.dma_start(
                    wqkv_sb[:, s, 2 * C : 3 * C], wq_r[:, s, 2 * C : 3 * C]
                )

            # ---------------- constants ----------------
            # causal multiplicative mask for the transposed diagonal block:
            # cmask[k, q] = 1 if q >= k else 0
            cmask = big.tile([P, P], bf16, tag="cmask")
            nc.gpsimd.memset(cmask, 1.0)
            nc.gpsimd.affine_select(
                out=cmask,
                in_=cmask,
                compare_op=ALU.is_ge,
                fill=0.0,
                base=0,
                pattern=[[1, P]],
                channel_multiplier=-1,
            )
            # per-partition bias columns for the Q/K part of qkvT
            bqk_col = big.tile([P, 2 * C // P], f32, tag="bqk")
            nc.gpsimd.dma_start(bqk_col, bqkv_d[: 2 * C].rearrange("(o p) -> p o", p=P))
            # broadcast bias rows (per free-dim column) for V and proj
            bias_v = big.tile([P, C], f32, tag="bias_v")
            nc.gpsimd.dma_start(bias_v, bqkv_d[2 * C :][None, :].to_broadcast((P, C)))
            # ones column [1, HD] for the last-pair reciprocal broadcast
            ones_col = big.tile([1, HD], bf16, tag="ones_col")
            nc.gpsimd.memset(ones_col, 1.0)

            wproj_sb = big.tile([P, NS, C], bf16, tag="wproj")
            wp_r = wproj_d[:, :].rearrange("(s p) j -> p s j", p=P)

            # ---------------- Q/K^T tiles (interleaved with attention) ----
            qkt_sb = big.tile([P, 2 * C // P, T], bf16, tag="qkt")

            def emit_qk(m):
                # ch-inner so both matmuls share one LDWEIGHTS per s
                ps0 = pmm.tile([P, 512], f32, tag="pmm", name=f"qk{m}_0")
                ps1 = pmm.tile([P, 512], f32, tag="pmm", name=f"qk{m}_1")
                for s in range(NS):
                    for ch, ps in ((0, ps0), (1, ps1)):
                        nc.tensor.matmul(
                            ps,
                            wqkv_sb[:, s, m * P : (m + 1) * P],
                            xt_sb[:, s, ch * 512 : (ch + 1) * 512],
                            start=(s == 0),
                            stop=(s == NS - 1),
                        )
                nc.vector.tensor_scalar_add(
                    qkt_sb[:, m, 0:512], ps0, bqk_col[:, m : m + 1]
                )
                nc.vector.tensor_scalar_add(
                    qkt_sb[:, m, 512:T], ps1, bqk_col[:, m : m + 1]
                )

            # pair 0's Q/K first so ScalarE's exp pipeline starts early
            emit_qk(0)
            emit_qk(C // P)

            # ---------------- V (natural layout, ones-augmented) ----------
            v_sb = [
                big.tile([P, H * W], bf16, tag=f"v{i}", name=f"v{i}") for i in range(NT)
            ]

            def emit_v(i):
                v3 = v_sb[i].rearrange("p (h w) -> p h w", w=W)
                nc.gpsimd.memset(v3[:, :, HD : HD + 1], 1.0)
                ps0 = pmm.tile([P, 512], f32, tag="pmm", name=f"v{i}_0")
                ps1 = pmm.tile([P, 512], f32, tag="pmm", name=f"v{i}_1")
                for s in range(NS):
                    for ch, ps in ((0, ps0), (1, ps1)):
                        nc.tensor.matmul(
                            ps,
                            xt_sb[:, s, i * P : (i + 1) * P],
                            wqkv_sb[:, s, 2 * C + ch * 512 : 2 * C + (ch + 1) * 512],
                            start=(s == 0),
                            stop=(s == NS - 1),
                        )
                for ch, ps in ((0, ps0), (1, ps1)):
                    nc.vector.tensor_tensor(
                        v3[:, 8 * ch : 8 * ch + 8, 0:HD],
                        ps.rearrange("p (h d) -> p h d", d=HD),
                        bias_v[:, ch * 512 : (ch + 1) * 512].rearrange(
                            "p (h d) -> p h d", d=HD
                        ),
                        ALU.add,
                    )

            # ---------------- attention ----------------
            yt_sb = [
                big.tile([P, T], bf16, tag=f"yt{g}", name=f"yt{g}") for g in range(NT)
            ]

            def s_matmuls(sp0, sp1, kt_h0, qt_h0, kt_h1, qt_h1, kt):
                # interleave the two heads' matmuls so they occupy
                # different PE row groups concurrently
                q0 = kt * P
                if kt <= 3:
                    spans = [(q0, 512), (512, T)]
                else:
                    spans = [(q0, T)]
                for lo, hi in spans:
                    nc.tensor.matmul(
                        sp0[:, lo:hi],
                        kt_h0[:, q0 : q0 + P],
                        qt_h0[:, lo:hi],
                        start=True,
                        stop=True,
                        tile_position=(0, 0),
                    )
                    nc.tensor.matmul(
                        sp1[:, lo:hi],
                        kt_h1[:, q0 : q0 + P],
                        qt_h1[:, lo:hi],
                        start=True,
                        stop=True,
                        tile_position=(64, 0),
                    )

            def av_matmuls(ypA, ypB, pt_ap, vcols, kt, q_off):
                # ypA covers q columns [0,512), ypB [512,T); pt_ap covers
                # q columns [q_off, T); accumulate over kt
                q0 = kt * P
                lhsT_v = v_sb[kt][:, vcols : vcols + HD + 1]  # [128, 65]
                if kt <= 3:
                    nc.tensor.matmul(
                        ypA[0 : HD + 1, q0:512],
                        lhsT_v,
                        pt_ap[:, q0 - q_off : 512 - q_off],
                        start=(kt == 0),
                        stop=(kt == 3),
                    )
                    nc.tensor.matmul(
                        ypB[0 : HD + 1, 0:512],
                        lhsT_v,
                        pt_ap[:, 512 - q_off : T - q_off],
                        start=(kt == 0),
                        stop=(kt == NT - 1),
                    )
                else:
                    nc.tensor.matmul(
                        ypB[0 : HD + 1, q0 - 512 : 512],
                        lhsT_v,
                        pt_ap[:, q0 - q_off : T - q_off],
                        start=False,
                        stop=(kt == NT - 1),
                    )

            def evac_head(ypA, ypB, h, fast):
                # evacuate unnormalized Y + sums immediately to free the Y
                # psum tile; split ACT/DVE halves so neither engine gates.
                yu = small.tile([HD + 1, T], bf16, tag="yu", name=f"yu{h}")
                nc.scalar.copy(yu[:, 0:512], ypA[0 : HD + 1, 0:512])
                nc.vector.tensor_copy(yu[:, 512:T], ypB[0 : HD + 1, 0:512])
                if fast:
                    return (yu, h, True)
                # start the reciprocal DMA chain (reblock via DRAM)
                dma = nc.gpsimd.dma_start
                scr = dramp.tile([T], bf16, tag="scr", name=f"scr{h}")
                dma(scr[None, :], yu[HD : HD + 1, :])
                s64 = small1.tile([HD, T // HD], bf16, tag="s64", name=f"s64_{h}")
                dma(s64, scr.rearrange("(p e) -> p e", p=HD))
                return (yu, h, False, s64)

            def norm_head(state):
                # reciprocal + broadcast + normalize; emitted ~a pair later
                # so the DVE never head-of-line blocks on the DMA chain
                yu, h, fast = state[:3]
                g = h // 2
                if fast:
                    # DVE recip on the sums row + PE broadcast (no DRAM hop)
                    rrow = small1.tile([1, T], bf16, tag="rrow", name=f"rrow{h}")
                    with nc.allow_low_precision("softmax recips in bf16 (tol 2e-2)"):
                        nc.vector.reciprocal(rrow, yu[HD : HD + 1, :])
                    rb0 = pmm.tile([HD, 512], f32, tag="pmm", name=f"rb{h}_0")
                    rb1 = pmm.tile([HD, 512], f32, tag="pmm", name=f"rb{h}_1")
                    nc.tensor.matmul(rb0, ones_col, rrow[:, 0:512], start=True, stop=True)
                    nc.tensor.matmul(rb1, ones_col, rrow[:, 512:T], start=True, stop=True)
                    if h % 2 == 0:
                        nc.vector.tensor_tensor(
                            yt_sb[g][0:HD, 0:512], yu[0:HD, 0:512], rb0, ALU.mult
                        )
                        nc.vector.tensor_tensor(
                            yt_sb[g][0:HD, 512:T], yu[0:HD, 512:T], rb1, ALU.mult
                        )
                    else:
                        ytmp = small1.tile([HD, T], bf16, tag="ytmp", name=f"ytmp{h}")
                        nc.vector.tensor_tensor(
                            ytmp[:, 0:512], yu[0:HD, 0:512], rb0, ALU.mult
                        )
                        nc.vector.tensor_tensor(
                            ytmp[:, 512:T], yu[0:HD, 512:T], rb1, ALU.mult
                        )
                        nc.sync.dma_start(yt_sb[g][HD:P, :], ytmp)
                    return
                s64 = state[3]
                dma = nc.gpsimd.dma_start
                r64 = small1.tile([HD, T // HD], bf16, tag="r64", name=f"r64_{h}")
                with nc.allow_low_precision("softmax recips in bf16 (tol 2e-2)"):
                    nc.vector.reciprocal(r64, s64)
                scr2 = dramp.tile([T], bf16, tag="scr2", name=f"scr2_{h}")
                dma(scr2.rearrange("(p e) -> p e", p=HD), r64)
                r_sb = small.tile([HD, T], bf16, tag="r", name=f"r{h}")
                dma(r_sb, scr2[None, :].to_broadcast((HD, T)))
                if h % 2 == 0:
                    nc.vector.tensor_tensor(yt_sb[g][0:HD, :], yu[0:HD, :], r_sb, ALU.mult)
                else:
                    ytmp = small1.tile([HD, T], bf16, tag="ytmp", name=f"ytmp{h}")
                    nc.vector.tensor_tensor(ytmp, yu[0:HD, :], r_sb, ALU.mult)
                    # partition shift 0..63 -> 64..127 via SBUF-to-SBUF DMA
                    dma(yt_sb[g][HD:P, :], ytmp)

            with (
                tc.tile_pool(name="psp", bufs=2, space="PSUM") as psp,
                tc.tile_pool(name="pyp", bufs=1, space="PSUM") as pyp,
            ):
                pending = []
                for g in range(NT):
                    # pair 7 swaps roles so the no-shift (even) head lands last
                    swap = g == NT - 1
                    h_on, h_def = (2 * g + 1, 2 * g) if swap else (2 * g, 2 * g + 1)
                    m = g
                    if g > 0:
                        emit_qk(m)
                        emit_qk((C // P) + m)
                    if g == 2:
                        # wproj load emitted early enough to overlap attention
                        for s in range(NS):
                            nc.sync.dma_start(wproj_sb[:, s, :], wp_r[:, s, :])
                    sl_on = (HD, P) if swap else (0, HD)
                    sl_def = (0, HD) if swap else (HD, P)
                    qt_on = qkt_sb[sl_on[0] : sl_on[1], m, :]
                    kt_on = qkt_sb[sl_on[0] : sl_on[1], (C // P) + m, :]
                    qt_def = qkt_sb[sl_def[0] : sl_def[1], m, :]
                    kt_def = qkt_sb[sl_def[0] : sl_def[1], (C // P) + m, :]
                    tp_on = (sl_on[0], 0)
                    tp_def = (sl_def[0], 0)
                    yp = pyp.tile([P, T], f32, tag="py", name=f"yp{h_on}")
                    ypA, ypB = yp[:, 0:512], yp[:, 512:T]
                    pt_defs = []
                    for kt in range(NT):
                        if g == 0:
                            emit_v(kt)
                        if kt in (2, 5) and pending:
                            norm_head(pending.pop(0))
                        q0 = kt * P
                        sp_on = psp.tile([P, T], f32, tag="ps", name=f"spA_{g}_{kt}")
                        sp_def = psp.tile([P, T], f32, tag="ps", name=f"spB_{g}_{kt}")
                        if kt <= 3:
                            spans = [(q0, 512), (512, T)]
                        else:
                            spans = [(q0, T)]
                        for lo, hi in spans:
                            nc.tensor.matmul(
                                sp_on[:, lo:hi],
                                kt_on[:, q0 : q0 + P],
                                qt_on[:, lo:hi],
                                start=True,
                                stop=True,
                                tile_position=tp_on,
                            )
                            nc.tensor.matmul(
                                sp_def[:, lo:hi],
                                kt_def[:, q0 : q0 + P],
                                qt_def[:, lo:hi],
                                start=True,
                                stop=True,
                                tile_position=tp_def,
                            )
                        pt_on = ptp.tile([P, T], bf16, tag="pt", name=f"ptA_{g}_{kt}")
                        nc.scalar.activation(
                            pt_on[:, q0:T], sp_on[:, q0:T], AF.Exp, scale=0.125
                        )
                        pt_def = small1.tile(
                            [P, T - q0], bf16, tag=f"ptB_{kt}", name=f"ptB_{g}_{kt}"
                        )
                        nc.scalar.activation(pt_def, sp_def[:, q0:T], AF.Exp, scale=0.125)
                        # mask the diagonal block (k > q within the block -> 0)
                        nc.vector.tensor_tensor(
                            pt_on[:, q0 : q0 + P], pt_on[:, q0 : q0 + P], cmask, ALU.mult
                        )
                        nc.vector.tensor_tensor(
                            pt_def[:, 0:P], pt_def[:, 0:P], cmask, ALU.mult
                        )
                        av_matmuls(ypA, ypB, pt_on, h_on * W, kt, 0)
                        pt_defs.append(pt_def)
                    pending.append(evac_head(ypA, ypB, h_on, fast=(g >= NT - 1)))
                    yp1 = pyp.tile([P, T], f32, tag="py", name=f"yp{h_def}")
                    yp1A, yp1B = yp1[:, 0:512], yp1[:, 512:T]
                    for kt in range(NT):
                        av_matmuls(yp1A, yp1B, pt_defs[kt], h_def * W, kt, kt * P)
                    pending.append(evac_head(yp1A, yp1B, h_def, fast=(g >= NT - 1)))

                while pending:
                    norm_head(pending.pop(0))

            # ---------------- output projection ----------------
            # reuse the V bias tile for the proj bias (V phase is done)
            bias_o = bias_v
            nc.gpsimd.dma_start(bias_o, bproj_d[:][None, :].to_broadcast((P, C)))
            out_r = out_d[:, :].rearrange("(i p) j -> p i j", p=P)
            for i in range(NT):
                ps0 = pmm.tile([P, 512], f32, tag="pmm", name=f"proj{i}_0")
                ps1 = pmm.tile([P, 512], f32, tag="pmm", name=f"proj{i}_1")
                for g in range(NT):
                    for ch, ps in ((0, ps0), (1, ps1)):
                        nc.tensor.matmul(
                            ps,
                            yt_sb[g][:, i * P : (i + 1) * P],
                            wproj_sb[:, g, ch * 512 : (ch + 1) * 512],
                            start=(g == 0),
                            stop=(g == NT - 1),
                        )
                for ch, ps in ((0, ps0), (1, ps1)):
                    ot = outp.tile([P, 512], f32, tag="out")
                    nc.vector.tensor_tensor(
                        ot, ps, bias_o[:, ch * 512 : (ch + 1) * 512], ALU.add
                    )
                    nc.sync.dma_start(out_r[:, i, ch * 512 : (ch + 1) * 512], ot)

    nc.compile()
    return nc


_NC = None


def _get_nc():
    global _NC
    if _NC is None:
        _NC = _build()
    return _NC


def _in_maps(x, Wqkv, bqkv, Wproj, bproj):
    bf = ml_dtypes.bfloat16
    x = np.ascontiguousarray(np.asarray(x, dtype=np.float32).astype(bf))
    shared = {
        "wqkv": np.ascontiguousarray(np.asarray(Wqkv, dtype=np.float32).astype(bf)),
        "bqkv": np.ascontiguousarray(np.asarray(bqkv, dtype=np.float32)),
        "wproj": np.ascontiguousarray(np.asarray(Wproj, dtype=np.float32).astype(bf)),
        "bproj": np.ascontiguousarray(np.asarray(bproj, dtype=np.float32)),
    }
    return [{"x": np.ascontiguousarray(x[b]), **shared} for b in range(B)]


def run(x, Wqkv, bqkv, Wproj, bproj, **run_kwargs):
    """Run on 8 cores; returns (output [B,T,C] fp32, BassKernelResults)."""
    nc = _get_nc()
    res = run_bass_kernel_spmd(
        nc, _in_maps(x, Wqkv, bqkv, Wproj, bproj), core_ids=list(range(B)), **run_kwargs
    )
    out = np.stack([res.results[b]["out"] for b in range(B)]).astype(np.float32)
    return out, res


def kernel(x, Wqkv, bqkv, Wproj, bproj, n_head=None, **_ignored):
    out, _ = run(x, Wqkv, bqkv, Wproj, bproj)
    return out


# revision 11
# speedup vs baseline: 1.0707x; 1.0707x over previous
"""Causal self-attention (B=8, T=1024, C=1024, H=16) on 8 TRN2 NeuronCores.

Sharding: pure data-parallel over batch - core b computes batch element b
with fully replicated weights (B == n_cores, so no collectives needed).

v2 vs baseline:
  - x / Wqkv / Wproj are cast to bf16 on the HOST, so no on-device weight
    casts and half the weight DMA traffic.
  - x is transposed by the DMA xbar (dma_start_transpose) straight from
    DRAM into SBUF - no PE transposes, no PSUM evacuation copies.
  - QKV / V / proj matmul loops are ordered so consecutive matmuls share
    the stationary operand (halves LDWEIGHTS traffic).
  - Score matmuls for the head pair are emitted interleaved with explicit
    tile_position (0,0)/(64,0) so the PE can run both 64-contraction
    matmuls concurrently in different row groups.
  - Y evacuation split across ACT+DVE; V/proj-bias evacuations on Pool;
    softmax reciprocal chain batched per head with a DMA reblock; the
    last pair uses a DVE-recip + PE-broadcast fast path to cut the tail.
"""

import numpy as np
import ml_dtypes

import concourse.tile as tile
from concourse import bacc, mybir
from concourse.bass_utils import run_bass_kernel_spmd
from concourse.masks import make_identity

f32 = mybir.dt.float32
bf16 = mybir.dt.bfloat16
AF = mybir.ActivationFunctionType
ALU = mybir.AluOpType

B, T, C, H, HD = 8, 1024, 1024, 16, 64
P = 128
NT = T // P  # 8 token tiles
NS = C // P  # 8 contraction subtiles
W = 66  # per-head stride in V_sb: [64 vals][1 ones][1 pad]


def _build():
    nc = bacc.Bacc(trn_type="TRN2")
    x_d = nc.dram_tensor("x", (T, C), bf16, kind="ExternalInput")
    wqkv_d = nc.dram_tensor("wqkv", (C, 3 * C), bf16, kind="ExternalInput")
    bqkv_d = nc.dram_tensor("bqkv", (3 * C,), f32, kind="ExternalInput")
    wproj_d = nc.dram_tensor("wproj", (C, C), bf16, kind="ExternalInput")
    bproj_d = nc.dram_tensor("bproj", (C,), f32, kind="ExternalInput")
    out_d = nc.dram_tensor("out", (T, C), f32, kind="ExternalOutput")

    with tile.TileContext(nc) as tc:
        with (
            tc.tile_pool(name="big", bufs=1) as big,
            tc.tile_pool(name="ptp", bufs=4) as ptp,
            tc.tile_pool(name="small", bufs=3) as small,
            tc.tile_pool(name="small1", bufs=1) as small1,
            tc.tile_pool(name="outp", bufs=3) as outp,
            tc.tile_pool(name="dramp", bufs=6, space="DRAM") as dramp,
            tc.tile_pool(name="pmm", bufs=2, space="PSUM") as pmm,
        ):
            # ---------------- weights (bf16, no casts) ----------------
            # Q/K m-pair slices on the ACT ring so they overlap the x
            # transpose on the sync ring; ACT's ring is idle at startup
            # (first exp is ~15us in) and these issues finish well before.
            wqkv_sb = big.tile([P, NS, 3 * C], bf16, tag="wqkv")
            wq_r = wqkv_d[:, :].rearrange("(s p) i -> p s i", p=P)
            for mp in range(NS):
                nc.scalar.dma_start(
                    wqkv_sb[:, :, mp * 256 : (mp + 1) * 256],
                    wq_r[:, :, mp * 256 : (mp + 1) * 256],
                )

            # ---------------- x transpose via DMA xbar ----------------
            # xt[p, s, t] = x[t, s*128+p]; single instruction, no PE work.
            xt_sb = big.tile([P, NS, T], bf16, tag="xt")
            nc.sync.dma_start_transpose(xt_sb, x_d[:, :])

            # V columns per s-row (2KB descriptors)
            for s in range(NS):
                nc.sync.dma_start(
                    wqkv_sb[:, s, 2 * C : 3 * C], wq_r[:, s, 2 * C : 3 * C]
                )

            # ---------------- constants ----------------
            # causal multiplicative mask for the transposed diagonal block:
            # cmask[k, q] = 1 if q >= k else 0
            cmask = big.tile([P, P], bf16, tag="cmask")
            nc.gpsimd.memset(cmask, 1.0)
            nc.gpsimd.affine_select(
                out=cmask,
                in_=cmask,
                compare_op=ALU.is_ge,
                fill=0.0,
                base=0,
                pattern=[[1, P]],
                channel_multiplier=-1,
            )
            # per-partition bias columns for the Q/K part of qkvT
            bqk_col = big.tile([P, 2 * C // P], f32, tag="bqk")
            nc.gpsimd.dma_start(bqk_col, bqkv_d[: 2 * C].rearrange("(o p) -> p o", p=P))
            # broadcast bias rows (per free-dim column) for V and proj
            bias_v = big.tile([P, C], f32, tag="bias_v")
            nc.gpsimd.dma_start(bias_v, bqkv_d[2 * C :][None, :].to_broadcast((P, C)))
            wproj_sb = big.tile([P, NS, C], bf16, tag="wproj")
            wp_r = wproj_d[:, :].rearrange("(s p) j -> p s j", p=P)

            # ---------------- Q/K^T tiles (interleaved with attention) ----
            qkt_sb = big.tile([P, 2 * C // P, T], bf16, tag="qkt")

            def emit_qk(m):
                # ch-inner so both matmuls share one LDWEIGHTS per s
                ps0 = pmm.tile([P, 512], f32, tag="pmm", name=f"qk{m}_0")
                ps1 = pmm.tile([P, 512], f32, tag="pmm", name=f"qk{m}_1")
                for s in range(NS):
                    for ch, ps in ((0, ps0), (1, ps1)):
                        nc.tensor.matmul(
                            ps,
                            wqkv_sb[:, s, m * P : (m + 1) * P],
                            xt_sb[:, s, ch * 512 : (ch + 1) * 512],
                            start=(s == 0),
                            stop=(s == NS - 1),
                        )
                nc.vector.tensor_scalar_add(
                    qkt_sb[:, m, 0:512], ps0, bqk_col[:, m : m + 1]
                )
                nc.vector.tensor_scalar_add(
                    qkt_sb[:, m, 512:T], ps1, bqk_col[:, m : m + 1]
                )

            # pair 0's Q/K first so ScalarE's exp pipeline starts early
            emit_qk(0)
            emit_qk(C // P)

            # ---------------- V (natural layout, ones-augmented) ----------
            v_sb = [
                big.tile([P, H * W], bf16, tag=f"v{i}", name=f"v{i}") for i in range(NT)
            ]

            def emit_v(i):
                v3 = v_sb[i].rearrange("p (h w) -> p h w", w=W)
                nc.gpsimd.memset(v3[:, :, HD : HD + 1], 1.0)
                ps0 = pmm.tile([P, 512], f32, tag="pmm", name=f"v{i}_0")
                ps1 = pmm.tile([P, 512], f32, tag="pmm", name=f"v{i}_1")
                for s in range(NS):
                    for ch, ps in ((0, ps0), (1, ps1)):
                        nc.tensor.matmul(
                            ps,
                            xt_sb[:, s, i * P : (i + 1) * P],
                            wqkv_sb[:, s, 2 * C + ch * 512 : 2 * C + (ch + 1) * 512],
                            start=(s == 0),
                            stop=(s == NS - 1),
                        )
                for ch, ps in ((0, ps0), (1, ps1)):
                    nc.vector.tensor_tensor(
                        v3[:, 8 * ch : 8 * ch + 8, 0:HD],
                        ps.rearrange("p (h d) -> p h d", d=HD),
                        bias_v[:, ch * 512 : (ch + 1) * 512].rearrange(
                            "p (h d) -> p h d", d=HD
                        ),
                        ALU.add,
                    )

            # ---------------- attention ----------------
            yt_sb = [
                big.tile([P, T], bf16, tag=f"yt{g}", name=f"yt{g}") for g in range(NT)
            ]

            def s_matmuls(sp0, sp1, kt_h0, qt_h0, kt_h1, qt_h1, kt):
                # interleave the two heads' matmuls so they occupy
                # different PE row groups concurrently
                q0 = kt * P
                if kt <= 3:
                    spans = [(q0, 512), (512, T)]
                else:
                    spans = [(q0, T)]
                for lo, hi in spans:
                    nc.tensor.matmul(
                        sp0[:, lo:hi],
                        kt_h0[:, q0 : q0 + P],
                        qt_h0[:, lo:hi],
                        start=True,
                        stop=True,
                        tile_position=(0, 0),
                    )
                    nc.tensor.matmul(
                        sp1[:, lo:hi],
                        kt_h1[:, q0 : q0 + P],
                        qt_h1[:, lo:hi],
                        start=True,
                        stop=True,
                        tile_position=(64, 0),
                    )

            def av_matmuls(ypA, ypB, pt_ap, vcols, kt, q_off):
                # ypA covers q columns [0,512), ypB [512,T); pt_ap covers
                # q columns [q_off, T); accumulate over kt
                q0 = kt * P
                lhsT_v = v_sb[kt][:, vcols : vcols + HD + 1]  # [128, 65]
                if kt <= 3:
                    nc.tensor.matmul(
                        ypA[0 : HD + 1, q0:512],
                        lhsT_v,
                        pt_ap[:, q0 - q_off : 512 - q_off],
                        start=(kt == 0),
                        stop=(kt == 3),
                    )
                    nc.tensor.matmul(
                        ypB[0 : HD + 1, 0:512],
                        lhsT_v,
                        pt_ap[:, 512 - q_off : T - q_off],
                        start=(kt == 0),
                        stop=(kt == NT - 1),
                    )
                else:
                    nc.tensor.matmul(
                        ypB[0 : HD + 1, q0 - 512 : 512],
                        lhsT_v,
                        pt_ap[:, q0 - q_off : T - q_off],
                        start=False,
                        stop=(kt == NT - 1),
                    )

            def evac_headA(ypA, h):
                # the A half (q cols 0:512) finishes accumulating at kt=3,
                # so it is evacuated early, off the critical path
                yu = small.tile([HD + 1, T], bf16, tag="yu", name=f"yu{h}")
                nc.scalar.copy(yu[:, 0:512], ypA[0 : HD + 1, 0:512])
                return yu

            def evac_headB(yu, ypB, h, fast):
                # the B half gates the psum recycle: split ACT/DVE quarters
                nc.scalar.copy(yu[:, 512:768], ypB[0 : HD + 1, 0:256])
                nc.vector.tensor_copy(yu[:, 768:T], ypB[0 : HD + 1, 256:512])
                # start the reciprocal DMA chain (reblock via DRAM)
                dma = nc.sync.dma_start if fast else nc.gpsimd.dma_start
                scr = dramp.tile([T], bf16, tag="scr", name=f"scr{h}")
                dma(scr[None, :], yu[HD : HD + 1, :])
                s64 = small1.tile([HD, T // HD], bf16, tag="s64", name=f"s64_{h}")
                dma(s64, scr.rearrange("(p e) -> p e", p=HD))
                return (yu, h, fast, s64)

            def norm_head(state):
                # reciprocal + broadcast + normalize; emitted ~a pair later
                # so the DVE never head-of-line blocks on the DMA chain
                yu, h, fast, s64 = state
                g = h // 2
                dma = nc.sync.dma_start if fast else nc.gpsimd.dma_start
                r64 = small1.tile([HD, T // HD], bf16, tag="r64", name=f"r64_{h}")
                with nc.allow_low_precision("softmax recips in bf16 (tol 2e-2)"):
                    nc.vector.reciprocal(r64, s64)
                scr2 = dramp.tile([T], bf16, tag="scr2", name=f"scr2_{h}")
                dma(scr2.rearrange("(p e) -> p e", p=HD), r64)
                r_sb = small.tile([HD, T], bf16, tag="r", name=f"r{h}")
                dma(r_sb, scr2[None, :].to_broadcast((HD, T)))
                if h % 2 == 0:
                    nc.vector.tensor_tensor(yt_sb[g][0:HD, :], yu[0:HD, :], r_sb, ALU.mult)
                else:
                    ytmp = small1.tile([HD, T], bf16, tag="ytmp", name=f"ytmp{h}")
                    nc.vector.tensor_tensor(ytmp, yu[0:HD, :], r_sb, ALU.mult)
                    # partition shift 0..63 -> 64..127 via SBUF-to-SBUF DMA
                    dma(yt_sb[g][HD:P, :], ytmp)

            with (
                tc.tile_pool(name="psp", bufs=2, space="PSUM") as psp,
                tc.tile_pool(name="pyp", bufs=1, space="PSUM") as pyp,
            ):
                pending = []
                for g in range(NT):
                    # pair 7 swaps roles so the no-shift (even) head lands last
                    swap = g == NT - 1
                    h_on, h_def = (2 * g + 1, 2 * g) if swap else (2 * g, 2 * g + 1)
                    m = g
                    if g > 0:
                        emit_qk(m)
                        emit_qk((C // P) + m)
                    if g == 2:
                        # wproj load emitted early enough to overlap attention
                        for s in range(NS):
                            nc.sync.dma_start(wproj_sb[:, s, :], wp_r[:, s, :])
                    sl_on = (HD, P) if swap else (0, HD)
                    sl_def = (0, HD) if swap else (HD, P)
                    qt_on = qkt_sb[sl_on[0] : sl_on[1], m, :]
                    kt_on = qkt_sb[sl_on[0] : sl_on[1], (C // P) + m, :]
                    qt_def = qkt_sb[sl_def[0] : sl_def[1], m, :]
                    kt_def = qkt_sb[sl_def[0] : sl_def[1], (C // P) + m, :]
                    tp_on = (sl_on[0], 0)
                    tp_def = (sl_def[0], 0)
                    yp = pyp.tile([P, T], f32, tag="py", name=f"yp{h_on}")
                    ypA, ypB = yp[:, 0:512], yp[:, 512:T]
                    pt_defs = []
                    yu_on = None
                    for kt in range(NT):
                        if g == 0:
                            emit_v(kt)
                        if kt == 5:
                            yu_on = evac_headA(ypA, h_on)
                        if kt in (2, 5) and pending:
                            norm_head(pending.pop(0))
                        q0 = kt * P
                        sp_on = psp.tile([P, T], f32, tag="ps", name=f"spA_{g}_{kt}")
                        sp_def = psp.tile([P, T], f32, tag="ps", name=f"spB_{g}_{kt}")
                        if kt <= 3:
                            spans = [(q0, 512), (512, T)]
                        else:
                            spans = [(q0, T)]
                        for lo, hi in spans:
                            nc.tensor.matmul(
                                sp_on[:, lo:hi],
                                kt_on[:, q0 : q0 + P],
                                qt_on[:, lo:hi],
                                start=True,
                                stop=True,
                                tile_position=tp_on,
                            )
                            nc.tensor.matmul(
                                sp_def[:, lo:hi],
                                kt_def[:, q0 : q0 + P],
                                qt_def[:, lo:hi],
                                start=True,
                                stop=True,
                                tile_position=tp_def,
                            )
                        pt_on = ptp.tile([P, T], bf16, tag="pt", name=f"ptA_{g}_{kt}")
                        nc.scalar.activation(
                            pt_on[:, q0:T], sp_on[:, q0:T], AF.Exp, scale=0.125
                        )
                        pt_def = small1.tile(
                            [P, T - q0], bf16, tag=f"ptB_{kt}", name=f"ptB_{g}_{kt}"
                        )
                        nc.scalar.activation(pt_def, sp_def[:, q0:T], AF.Exp, scale=0.125)
                        # mask the diagonal block (k > q within the block -> 0)
                        nc.vector.tensor_tensor(
                            pt_on[:, q0 : q0 + P], pt_on[:, q0 : q0 + P], cmask, ALU.mult
                        )
                        nc.vector.tensor_tensor(
                            pt_def[:, 0:P], pt_def[:, 0:P], cmask, ALU.mult
                        )
                        av_matmuls(ypA, ypB, pt_on, h_on * W, kt, 0)
                        pt_defs.append(pt_def)
                    fast = g >= NT - 2
                    pending.append(evac_headB(yu_on, ypB, h_on, fast))
                    yp1 = pyp.tile([P, T], f32, tag="py", name=f"yp{h_def}")
                    yp1A, yp1B = yp1[:, 0:512], yp1[:, 512:T]
                    yu_def = None
                    for kt in range(NT):
                        av_matmuls(yp1A, yp1B, pt_defs[kt], h_def * W, kt, kt * P)
                        if kt == 4:
                            yu_def = evac_headA(yp1A, h_def)
                    pending.append(evac_headB(yu_def, yp1B, h_def, fast))

                while pending:
                    norm_head(pending.pop(0))

            # ---------------- output projection ----------------
            # reuse the V bias tile for the proj bias (V phase is done)
            bias_o = bias_v
            nc.gpsimd.dma_start(bias_o, bproj_d[:][None, :].to_broadcast((P, C)))
            out_r = out_d[:, :].rearrange("(i p) j -> p i j", p=P)
            with tc.tile_pool(name="pproj", bufs=4, space="PSUM") as pproj:
                for i in range(NT):
                    ps0 = pproj.tile([P, 512], f32, tag="pproj", name=f"proj{i}_0")
                    ps1 = pproj.tile([P, 512], f32, tag="pproj", name=f"proj{i}_1")
                    for g in range(NT):
                        for ch, ps in ((0, ps0), (1, ps1)):
                            nc.tensor.matmul(
                                ps,
                                yt_sb[g][:, i * P : (i + 1) * P],
                                wproj_sb[:, g, ch * 512 : (ch + 1) * 512],
                                start=(g == 0),
                                stop=(g == NT - 1),
                            )
                    for ch, ps in ((0, ps0), (1, ps1)):
                        ot = outp.tile([P, 512], f32, tag="out")
                        nc.vector.tensor_tensor(
                            ot, ps, bias_o[:, ch * 512 : (ch + 1) * 512], ALU.add
                        )
                        nc.sync.dma_start(out_r[:, i, ch * 512 : (ch + 1) * 512], ot)

    nc.compile()
    return nc


_NC = None


def _get_nc():
    global _NC
    if _NC is None:
        _NC = _build()
    return _NC


def _in_maps(x, Wqkv, bqkv, Wproj, bproj):
    bf = ml_dtypes.bfloat16
    x = np.ascontiguousarray(np.asarray(x, dtype=np.float32).astype(bf))
    shared = {
        "wqkv": np.ascontiguousarray(np.asarray(Wqkv, dtype=np.float32).astype(bf)),
        "bqkv": np.ascontiguousarray(np.asarray(bqkv, dtype=np.float32)),
        "wproj": np.ascontiguousarray(np.asarray(Wproj, dtype=np.float32).astype(bf)),
        "bproj": np.ascontiguousarray(np.asarray(bproj, dtype=np.float32)),
    }
    return [{"x": np.ascontiguousarray(x[b]), **shared} for b in range(B)]


def run(x, Wqkv, bqkv, Wproj, bproj, **run_kwargs):
    """Run on 8 cores; returns (output [B,T,C] fp32, BassKernelResults)."""
    nc = _get_nc()
    res = run_bass_kernel_spmd(
        nc, _in_maps(x, Wqkv, bqkv, Wproj, bproj), core_ids=list(range(B)), **run_kwargs
    )
    out = np.stack([res.results[b]["out"] for b in range(B)]).astype(np.float32)
    return out, res


def kernel(x, Wqkv, bqkv, Wproj, bproj, n_head=None, **_ignored):
    out, _ = run(x, Wqkv, bqkv, Wproj, bproj)
    return out
